# revision 4
# baseline (speedup 1.0000x reference)
"""CARAFE content-aware upsampling kernel for 8 Trainium2 NeuronCores.

Problem: x (4,256,64,64) f32 -> out (4,256,128,128) f32.
  comp = 1x1 conv (256->64), BN(eval)+SiLU, 3x3 conv (64->100),
  softmax over 25 taps, per-pixel 5x5 weighted reassembly at 2x upscale.

Sharding: pure data parallel, 8 shards = 4 batches x 2 row-halves (32 rows),
with a 2-row halo handled by host-side padding.

Per-core dataflow (SPMD, identical program, per-core data):
  PE : compression conv (K=256 via 2 matmuls), 3x3 encoder conv as 9
       PSUM-accumulated matmuls, mask transpose via permuted-identity matmul
       (reorders channels k*4+s -> s*25+k while transposing to pixel-major).
  ACT: SiLU (BN shift folded into bias), Exp (softmax, no max-sub needed:
       logits are bounded), PSUM->SBUF mask copies.
  DVE: validity masking, softmax denominator (reduce over 25 taps) +
       reciprocal, and the main per-pixel reassembly: for each of 25 taps x
       4 subpixels one fused scalar_tensor_tensor (acc = x_shift*m + acc)
       where the mask value is a per-partition scalar (pixels on partitions).
       Softmax normalization is folded into the epilogue (multiply by 1/Z).
  DMA: x enters twice (channel-major for convs, pixel-major transposed for
       reassembly slabs); output leaves pixel-major, host un-transposes.
"""

import numpy as np

B, C, H, W = 4, 256, 64, 64
COMP = 64
SCALE, K_UP, K_ENC = 2, 5, 3
EPS = 1e-5
NCORES = 8
HS = H // 2          # 32 rows per core
PR = HS + 4          # 36 padded rows per shard
PCW = W + 4          # 68 padded cols
NPIX = HS * W        # 2048 output-res pixels per core
NACT = (HS + 2) * PCW  # 34*68 = 2312 act pixels (1-row halo for 3x3 conv)
NT = NPIX // 128     # 16 reassembly tiles (2 image rows each)

_cache = {}


def _build():
    from contextlib import ExitStack

    import concourse.bacc as bacc
    import concourse.bass as bass
    import concourse.mybir as mybir
    import concourse.tile as tile

    f32 = mybir.dt.float32
    nc = bacc.Bacc("TRN2", target_bir_lowering=False, debug=False,
                   num_devices=NCORES)

    xc = nc.dram_tensor("xc", (2, 128, PR * PCW), f32, kind="ExternalInput").ap()
    xt = nc.dram_tensor("xt", (PR * PCW, C), f32, kind="ExternalInput").ap()
    w_eff = nc.dram_tensor("w_eff", (2, 128, COMP), f32, kind="ExternalInput").ap()
    b_eff = nc.dram_tensor("b_eff", (COMP, 1), f32, kind="ExternalInput").ap()
    w_enc9 = nc.dram_tensor("w_enc9", (COMP, 9 * 100), f32, kind="ExternalInput").ap()
    perm = nc.dram_tensor("perm", (100, 100), f32, kind="ExternalInput").ap()
    vmask = nc.dram_tensor("vmask", (NACT,), f32, kind="ExternalInput").ap()
    out_t = nc.dram_tensor("out_t", (NPIX, 4, C), f32, kind="ExternalOutput").ap()

    mult = mybir.AluOpType.mult
    add = mybir.AluOpType.add
    AF = mybir.ActivationFunctionType

    with tile.TileContext(nc) as tc, ExitStack() as ctx:
        const = ctx.enter_context(tc.tile_pool(name="const", bufs=1))
        work = ctx.enter_context(tc.tile_pool(name="work", bufs=2))
        psA = ctx.enter_context(tc.tile_pool(name="psA", bufs=2, space="PSUM"))
        psB = ctx.enter_context(tc.tile_pool(name="psB", bufs=2, space="PSUM"))
        psC = ctx.enter_context(tc.tile_pool(name="psC", bufs=2, space="PSUM"))

        # ---- resident constants ----
        xc_s = []
        for h in range(2):
            t = const.tile([128, PR * PCW], f32, tag=f"xc{h}")
            nc.sync.dma_start(out=t, in_=xc[h])
            xc_s.append(t)
        w_eff_s = []
        for h in range(2):
            t = const.tile([128, COMP], f32, tag=f"weff{h}")
            nc.sync.dma_start(out=t, in_=w_eff[h])
            w_eff_s.append(t)
        b_eff_s = const.tile([COMP, 1], f32, tag="beff")
        nc.sync.dma_start(out=b_eff_s, in_=b_eff)
        w_enc_s = const.tile([COMP, 9 * 100], f32, tag="wenc")
        nc.sync.dma_start(out=w_enc_s, in_=w_enc9)
        perm_s = const.tile([100, 100], f32, tag="perm")
        nc.sync.dma_start(out=perm_s, in_=perm)
        vm_s = const.tile([COMP, NACT], f32, tag="vm")
        nc.sync.dma_start(
            out=vm_s,
            in_=bass.AP(tensor=vmask.tensor, offset=vmask.offset,
                        ap=[[0, COMP]] + list(vmask.ap)),
        )
        act_s = const.tile([COMP, NACT], f32, tag="act")
        act3 = act_s[:].rearrange("p (r c) -> p r c", c=PCW)

        # ---- compression conv + BN + SiLU + validity mask ----
        # act rows cover shard-local padded rows 1..35 (34 rows, 68 cols).
        for ci in range((NACT + 511) // 512):
            n0 = ci * 512
            n = min(512, NACT - n0)
            pc = psA.tile([COMP, 512], f32, tag="pc")
            for h in range(2):
                nc.tensor.matmul(
                    pc[:, :n], w_eff_s[h],
                    xc_s[h][:, PCW + n0:PCW + n0 + n],
                    start=(h == 0), stop=(h == 1),
                )
            sg = work.tile([COMP, 512], f32, tag="sg")
            nc.scalar.activation(out=sg[:, :n], in_=pc[:, :n],
                                 func=AF.Sigmoid, bias=b_eff_s, scale=1.0)
            # act = (comp + shift) * sigmoid(comp + shift), then validity mask
            nc.vector.scalar_tensor_tensor(
                out=act_s[:, n0:n0 + n], in0=pc[:, :n], scalar=b_eff_s,
                in1=sg[:, :n], op0=add, op1=mult)
            nc.vector.tensor_mul(act_s[:, n0:n0 + n], act_s[:, n0:n0 + n],
                                 vm_s[:, n0:n0 + n])

        # ---- per 8-row chunk: encoder conv, exp, transpose, denominators;
        #      then per 2-row tile: reassembly ----
        for mi in range(4):
            i0 = mi * 8
            pm = psB.tile([100, 512], f32, tag="pm")
            for idx in range(9):
                ky, kx = divmod(idx, 3)
                rhs = act3[:, i0 + ky:i0 + ky + 8, kx + 1:kx + 65]
                nc.tensor.matmul(
                    pm, w_enc_s[:, idx * 100:(idx + 1) * 100], rhs,
                    start=(idx == 0), stop=(idx == 8),
                )
            exp_s = work.tile([100, 512], f32, tag="exp")
            nc.scalar.activation(out=exp_s, in_=pm, func=AF.Exp)

            for q in range(4):
                t = mi * 4 + q
                pt = psC.tile([128, 100], f32, tag="pt")
                nc.tensor.matmul(pt, exp_s[:, q * 128:(q + 1) * 128], perm_s,
                                 start=True, stop=True)
                mk = work.tile([128, 100], f32, tag="mk", bufs=4)
                nc.scalar.activation(out=mk, in_=pt, func=AF.Copy)
                zs = work.tile([128, 4], f32, tag="zs", bufs=4)
                nc.vector.reduce_sum(
                    out=zs, in_=pt[:].rearrange("p (s k) -> p s k", k=25),
                    axis=mybir.AxisListType.X,
                )
                rz = work.tile([128, 4], f32, tag="rz", bufs=4)
                nc.vector.reciprocal(rz, zs)

                # 25 shifted pixel-major slabs for this 2-row tile
                xt3 = xt.rearrange("(r c) d -> r c d", c=PCW)
                slabs = []
                for k25 in range(25):
                    dy, dx = divmod(k25, 5)
                    R = work.tile([128, C], f32, tag="slab", bufs=52)
                    nc.sync.dma_start(out=R[0:64],
                                      in_=xt3[2 * t + dy, dx:dx + 64, :])
                    nc.sync.dma_start(out=R[64:128],
                                      in_=xt3[2 * t + dy + 1, dx:dx + 64, :])
                    slabs.append(R)

                acc = work.tile([128, 4, C], f32, tag="acc")
                for s in range(4):
                    nc.vector.tensor_scalar_mul(
                        out=acc[:, s], in0=slabs[0], scalar1=mk[:, s * 25:s * 25 + 1]
                    )
                    for k25 in range(1, 25):
                        nc.vector.scalar_tensor_tensor(
                            out=acc[:, s], in0=slabs[k25],
                            scalar=mk[:, s * 25 + k25:s * 25 + k25 + 1],
                            in1=acc[:, s], op0=mult, op1=add,
                        )
                ob = work.tile([128, 4, C], f32, tag="ob")
                for s in range(4):
                    nc.vector.tensor_scalar_mul(out=ob[:, s], in0=acc[:, s],
                                                scalar1=rz[:, s:s + 1])
                nc.sync.dma_start(out=out_t[t * 128:(t + 1) * 128], in_=ob)

    nc.compile()
    return nc


def _host_inputs(x, w_comp, bn_gamma, bn_beta, bn_mean, bn_var, w_enc):
    inv = (bn_gamma / np.sqrt(bn_var + EPS)).astype(np.float32)
    w_eff = (w_comp * inv[:, None]).T.astype(np.float32)          # (256,64)
    w_eff = np.ascontiguousarray(w_eff.reshape(2, 128, COMP))
    b_eff = (bn_beta - bn_mean * inv).astype(np.float32).reshape(COMP, 1)
    w_enc9 = np.ascontiguousarray(
        w_enc.transpose(1, 2, 3, 0).reshape(COMP, 9 * 100).astype(np.float32))
    perm = np.zeros((100, 100), np.float32)
    for k in range(25):
        for s in range(4):
            perm[k * 4 + s, s * 25 + k] = 1.0

    xp = np.pad(x.astype(np.float32), ((0, 0), (0, 0), (2, 2), (2, 2)))
    in_maps = []
    for core in range(NCORES):
        b, half = divmod(core, 2)
        h0 = HS * half
        sh = xp[b, :, h0:h0 + PR, :]                              # (256,36,68)
        xc = np.ascontiguousarray(sh.reshape(2, 128, PR * PCW))
        xt = np.ascontiguousarray(sh.transpose(1, 2, 0).reshape(PR * PCW, C))
        ar = h0 - 1 + np.arange(HS + 2)
        vr = (ar >= 0) & (ar < H)
        acj = np.arange(PCW) - 2
        vc = (acj >= 0) & (acj < W)
        vmask = (vr[:, None] & vc[None, :]).astype(np.float32).reshape(NACT)
        in_maps.append({"xc": xc, "xt": xt, "w_eff": w_eff, "b_eff": b_eff,
                        "w_enc9": w_enc9, "perm": perm, "vmask": vmask})
    return in_maps


def _run(nc, in_maps, **kw):
    from concourse import bass_utils
    return bass_utils.run_bass_kernel_spmd(nc, in_maps,
                                           core_ids=list(range(NCORES)), **kw)


def kernel(x, w_comp, bn_gamma, bn_beta, bn_mean, bn_var, w_enc):
    if "nc" not in _cache:
        _cache["nc"] = _build()
    in_maps = _host_inputs(np.asarray(x, np.float32), np.asarray(w_comp),
                           np.asarray(bn_gamma), np.asarray(bn_beta),
                           np.asarray(bn_mean), np.asarray(bn_var),
                           np.asarray(w_enc))
    res = _run(_cache["nc"], in_maps)
    out = np.zeros((B, C, H * SCALE, W * SCALE), np.float32)
    for core in range(NCORES):
        b, half = divmod(core, 2)
        h0 = HS * half
        ot = res.results[core]["out_t"]                           # (2048,4,256)
        o = (ot.reshape(HS, W, 2, 2, C).transpose(4, 0, 2, 1, 3)
               .reshape(C, HS * 2, W * 2))
        out[b, :, h0 * 2:h0 * 2 + HS * 2, :] = o
    return out
